# revision 18
# baseline (speedup 1.0000x reference)
"""Trainium2 Bass kernel for nn_ContrastiveSparsityLoss.

Strategy (8 NeuronCores, SPMD — all cores run one program on host-presliced data):
  Every (granularity) pair of samples is packed 2-wide into the PE partition dim
  (sample A ch 0..KC-1, sample B at KC..2KC-1) with row-shifted input copies so a
  single matmul covers up to 3 conv taps. Each core owns a 32-row band of every
  pair -> perfectly uniform work.

  Launch 1: conv3x3 (9/6/3 matmuls per 512-pix tile) -> y (bf16, core-local DRAM)
            + per-tile sum/sumsq accumulators (BN1 stats partials).
  host:     BN1 stats (f64), fold a1=rsqrt(var+eps) into w2' (gamma=1,beta=0).
  Launch 2: h1=relu(y-mu1) (DVE 4x), PE-transpose 128-chunks, M += h1T^T@h1T
            (second moment matrix -> BN2 stats algebraically, no stats pass over z).
  host:     mu_z = w2'@m1/cnt, var_z from diag(w2' M w2'^T), az=rsqrt(var_z+eps).
  Launch 3: z = w2'@h1 (PE), fused relu(z-mu_z)+GAP via activation/tensor_scalar
            accum_out. host: embeddings -> contrastive loss tail (f64).
"""
import os
import sys
import numpy as np
import ml_dtypes

sys.path.insert(0, '/opt/trn_rl_repo')

import concourse.bass as bass
import concourse.bacc as bacc
import concourse.tile as tile
from concourse import mybir
from concourse import bass_utils

BF16 = ml_dtypes.bfloat16
F32 = np.float32
AF = mybir.ActivationFunctionType
ALU = mybir.AluOpType

NCORES = 8
H = W = 256
EPS = 1e-5
TEMP = 0.1
NEG = -1e9

GS = ('g1', 'g2', 'g3')
CH = {'g1': 64, 'g2': 32, 'g3': 16}
COPIES = {'g1': 1, 'g2': 2, 'g3': 3}


def make_pairs():
    """15 pairs of (g, sidA, sidB); sid=(path, n), path 0=sparse 1=dense."""
    pairs = []
    for g in GS:
        ids = [(0, n) for n in range(5)] + [(1, n) for n in range(5)]
        for i in range(5):
            pairs.append((g, ids[2 * i], ids[2 * i + 1]))
    return pairs


PAIRS = make_pairs()


def mm_plan(g):
    """List of (roff, ccoff, taps) per matmul; taps = [(copy_k, w_row)]."""
    if g == 'g1':
        return [(1 + dy, 1 + dx, [(0, 1 + dy)])
                for dy in (-1, 0, 1) for dx in (-1, 0, 1)]
    if g == 'g2':
        return ([(0, 1 + dx, [(0, 0), (1, 1)]) for dx in (-1, 0, 1)]
                + [(2, 1 + dx, [(0, 2)]) for dx in (-1, 0, 1)])
    return [(0, 1 + dx, [(0, 0), (1, 1), (2, 2)]) for dx in (-1, 0, 1)]


# ---------------- host-side packing ----------------

def build_S_sample(x):
    """x [C,H,W] fp32 -> S [K*C, H+2, W+2] bf16 with baked row shifts."""
    C, h, w = x.shape
    g = {64: 'g1', 32: 'g2', 16: 'g3'}[C]
    K = COPIES[g]
    xb = x.astype(BF16)
    xpad = np.zeros((C, h + 2, w + 2), BF16)
    xpad[:, 1:-1, 1:-1] = xb
    S = np.zeros((K * C, h + 2, w + 2), BF16)
    for k in range(K):
        S[k * C:(k + 1) * C, :h + 2 - k, :] = xpad[:, k:, :]
    return S


def build_wblks(g, wA, wB):
    """-> [KP, nmm*128] bf16 stationary blocks."""
    C = CH[g]
    K = COPIES[g]
    KP = 2 * K * C
    plan = mm_plan(g)
    out = np.zeros((KP, len(plan) * 128), F32)
    for m, (roff, cc, taps) in enumerate(plan):
        dxw = cc  # = 1+dx, the w column index
        for (k, wrow) in taps:
            out[k * C:(k + 1) * C, m * 128:m * 128 + 64] = wA[:, :, wrow, dxw].T
            out[K * C + k * C:K * C + (k + 1) * C, m * 128 + 64:m * 128 + 128] = \
                wB[:, :, wrow, dxw].T
    return out.astype(BF16)


# ---------------- kernel builders ----------------

def new_nc(num_devices):
    return bacc.Bacc('TRN2', target_bir_lowering=False, debug=False,
                     enable_asserts=True, num_devices=num_devices)


def build_l1(nc, worklist, ntiles):
    """worklist: list of dicts {g}. Declares inputs S{p}, W{p}; outputs y, sacc."""
    np_ = len(worklist)
    Sin, Win = [], []
    for p, wd in enumerate(worklist):
        g = wd['g']
        KP = 2 * COPIES[g] * CH[g]
        nmm = len(mm_plan(g))
        Sin.append(nc.dram_tensor(f'S{p}', [KP, 2 * ntiles + 2, 258],
                                  mybir.dt.bfloat16, kind='ExternalInput').ap())
        Win.append(nc.dram_tensor(f'W{p}', [KP, nmm * 128],
                                  mybir.dt.bfloat16, kind='ExternalInput').ap())
    y_d = nc.dram_tensor('y', [np_, 128, ntiles * 512], mybir.dt.bfloat16,
                         kind='ExternalOutput').ap()
    sacc_d = nc.dram_tensor('sacc', [np_, 128, 32], mybir.dt.float32,
                            kind='ExternalOutput').ap()
    with tile.TileContext(nc) as tc:
        with tc.tile_pool(name='sband', bufs=2) as spool, \
             tc.tile_pool(name='wblk', bufs=2) as wpool, \
             tc.tile_pool(name='yst', bufs=4) as ypool, \
             tc.tile_pool(name='scr', bufs=2) as scrpool, \
             tc.tile_pool(name='sacc', bufs=2) as apool, \
             tc.tile_pool(name='psum', bufs=4, space='PSUM') as pspool:
            for p, wd in enumerate(worklist):
                g = wd['g']
                KP = 2 * COPIES[g] * CH[g]
                plan = mm_plan(g)
                nmm = len(plan)
                s_sb = spool.tile([128, 2 * ntiles + 2, 258], mybir.dt.bfloat16,
                                  tag='sband')
                nc.sync.dma_start(out=s_sb[0:KP], in_=Sin[p])
                w_sb = wpool.tile([128, 9 * 128], mybir.dt.bfloat16, tag='wblk')
                nc.sync.dma_start(out=w_sb[0:KP, 0:nmm * 128], in_=Win[p])
                sacc_sb = apool.tile([128, 32], mybir.dt.float32, tag='sacc')
                nc.vector.memset(sacc_sb[:], 0.0)
                for t in range(ntiles):
                    yps = pspool.tile([128, 512], mybir.dt.float32, tag='yps')
                    for m, (roff, cc, _) in enumerate(plan):
                        nc.tensor.matmul(
                            yps[:],
                            lhsT=w_sb[0:KP, m * 128:(m + 1) * 128],
                            rhs=s_sb[0:KP, roff + 2 * t:roff + 2 * t + 2,
                                     cc:cc + 256],
                            start=(m == 0), stop=(m == nmm - 1))
                    yst = ypool.tile([128, 512], mybir.dt.bfloat16, tag='yst')
                    nc.scalar.activation(yst[:], yps[:], AF.Copy,
                                         accum_out=sacc_sb[:, t:t + 1])
                    scr = scrpool.tile([128, 512], mybir.dt.bfloat16, tag='scr')
                    nc.vector.scalar_tensor_tensor(
                        scr[:], yst[:], 1.0, yst[:], ALU.mult, ALU.mult,
                        accum_out=sacc_sb[:, 16 + t:17 + t])
                    nc.sync.dma_start(out=y_d[p, :, t * 512:(t + 1) * 512],
                                      in_=yst[:])
                nc.sync.dma_start(out=sacc_d[p], in_=sacc_sb[:])
    nc.compile()
    return nc


def build_l2(nc, worklist, ntiles):
    np_ = len(worklist)
    y_d = nc.dram_tensor('y', [np_, 128, ntiles * 512], mybir.dt.bfloat16,
                         kind='ExternalInput').ap()
    bmu_d = nc.dram_tensor('bmu1', [128, np_], mybir.dt.float32,
                           kind='ExternalInput').ap()
    M_d = nc.dram_tensor('Mout', [np_, 128, 128], mybir.dt.float32,
                         kind='ExternalOutput').ap()
    m1_d = nc.dram_tensor('m1out', [np_, 128, 1], mybir.dt.float32,
                          kind='ExternalOutput').ap()
    h1_d = nc.dram_tensor('h1', [np_, 128, ntiles * 512], mybir.dt.bfloat16,
                          kind='ExternalOutput').ap()
    nch = ntiles * 4
    with tile.TileContext(nc) as tc:
        with tc.tile_pool(name='singles', bufs=1) as ones, \
             tc.tile_pool(name='yband', bufs=2) as ypool, \
             tc.tile_pool(name='h1', bufs=2) as hpool, \
             tc.tile_pool(name='ht', bufs=2) as htpool, \
             tc.tile_pool(name='small', bufs=3) as smpool, \
             tc.tile_pool(name='msb', bufs=2) as mpool, \
             tc.tile_pool(name='psm', bufs=2, space='PSUM') as psm:
            bmu_sb = ones.tile([128, np_], mybir.dt.float32)
            nc.sync.dma_start(out=bmu_sb[:], in_=bmu_d)
            zz_sb = ones.tile([128, ntiles * 512], mybir.dt.bfloat16)
            nc.vector.memset(zz_sb[:], 0.0)
            for p in range(np_):
                y_sb = ypool.tile([128, ntiles * 512], mybir.dt.bfloat16,
                                  tag='yband')
                nc.sync.dma_start(out=y_sb[:], in_=y_d[p])
                h1_sb = hpool.tile([128, ntiles * 512], mybir.dt.bfloat16,
                                   tag='h1')
                m1_sb = smpool.tile([128, 1], mybir.dt.float32, tag='m1')
                nc.vector.scalar_tensor_tensor(h1_sb[:], y_sb[:],
                                               bmu_sb[:, p:p + 1], zz_sb[:],
                                               ALU.add, ALU.max,
                                               accum_out=m1_sb[:])
                nc.scalar.dma_start(out=h1_d[p], in_=h1_sb[:])
                Mps = psm.tile([128, 128], mybir.dt.float32, tag='Mps')
                ht_sb = htpool.tile([128, nch, 128], mybir.dt.bfloat16,
                                    tag='ht')
                half = (nch // 2) * 128
                nc.sync.dma_start_transpose(
                    ht_sb[:, 0:nch // 2, :], h1_sb[:, 0:half])
                nc.scalar.dma_start_transpose(
                    ht_sb[:, nch // 2:nch, :], h1_sb[:, half:])
                for c in range(nch):
                    nc.tensor.matmul(
                        Mps[:], lhsT=ht_sb[:, c, :], rhs=ht_sb[:, c, :],
                        start=(c == 0), stop=(c == nch - 1))
                M_sb = mpool.tile([128, 128], mybir.dt.float32, tag='Msb')
                nc.vector.tensor_copy(M_sb[:], Mps[:])
                nc.sync.dma_start(out=M_d[p], in_=M_sb[:])
                nc.sync.dma_start(out=m1_d[p], in_=m1_sb[:])
    nc.compile()
    return nc


def build_l3(nc, worklist, ntiles):
    np_ = len(worklist)
    h1_d = nc.dram_tensor('h1', [np_, 128, ntiles * 512], mybir.dt.bfloat16,
                          kind='ExternalInput').ap()
    w2_d = nc.dram_tensor('w2p', [np_, 128, 256], mybir.dt.bfloat16,
                          kind='ExternalInput').ap()
    bmz_d = nc.dram_tensor('bmuz', [128, 2 * np_], mybir.dt.float32,
                           kind='ExternalInput').ap()
    g_d = nc.dram_tensor('gacc', [np_, 128, 32], mybir.dt.float32,
                         kind='ExternalOutput').ap()
    with tile.TileContext(nc) as tc:
        with tc.tile_pool(name='singles', bufs=1) as ones, \
             tc.tile_pool(name='h1', bufs=2) as hpool, \
             tc.tile_pool(name='w2', bufs=2) as wpool, \
             tc.tile_pool(name='scr', bufs=4) as scrpool, \
             tc.tile_pool(name='g', bufs=2) as gpool, \
             tc.tile_pool(name='psz', bufs=4, space='PSUM') as psz:
            bmz_sb = ones.tile([128, 2 * np_], mybir.dt.float32)
            nc.sync.dma_start(out=bmz_sb[:], in_=bmz_d)
            zz_sb = ones.tile([128, 512], mybir.dt.bfloat16)
            nc.vector.memset(zz_sb[:], 0.0)
            for p in range(np_):
                h1_sb = hpool.tile([128, ntiles * 512], mybir.dt.bfloat16,
                                   tag='h1')
                nc.sync.dma_start(out=h1_sb[:], in_=h1_d[p])
                w2_sb = wpool.tile([128, 256], mybir.dt.bfloat16, tag='w2')
                nc.sync.dma_start(out=w2_sb[:], in_=w2_d[p])
                g_sb = gpool.tile([128, 32], mybir.dt.float32, tag='g')
                nc.vector.memset(g_sb[:], 0.0)
                for t in range(ntiles):
                    zA = psz.tile([128, 512], mybir.dt.float32, tag='zA')
                    zB = psz.tile([128, 512], mybir.dt.float32, tag='zB')
                    rhs = h1_sb[:, t * 512:(t + 1) * 512]
                    nc.tensor.matmul(zA[:], lhsT=w2_sb[:, 0:128], rhs=rhs,
                                     start=True, stop=True)
                    nc.tensor.matmul(zB[:], lhsT=w2_sb[:, 128:256], rhs=rhs,
                                     start=True, stop=True)
                    scrA = scrpool.tile([128, 512], mybir.dt.bfloat16, tag='scrA')
                    nc.scalar.activation(scrA[:], zA[:], AF.Relu,
                                         bias=bmz_sb[:, 2 * p:2 * p + 1],
                                         scale=1.0,
                                         accum_out=g_sb[:, t:t + 1])
                    scrB = scrpool.tile([128, 512], mybir.dt.bfloat16, tag='scrB')
                    if t % 8 == 7:
                        # keep ACT/DVE balanced: ACT takes this B-branch too
                        nc.scalar.activation(scrB[:], zB[:], AF.Relu,
                                             bias=bmz_sb[:, 2 * p + 1:2 * p + 2],
                                             scale=1.0,
                                             accum_out=g_sb[:, 16 + t:17 + t])
                    else:
                        nc.vector.scalar_tensor_tensor(
                            scrB[:], zB[:], bmz_sb[:, 2 * p + 1:2 * p + 2],
                            zz_sb[:, 0:512], ALU.add, ALU.max,
                            accum_out=g_sb[:, 16 + t:17 + t])
                nc.sync.dma_start(out=g_d[p], in_=g_sb[:])
    nc.compile()
    return nc


# ---------------- host orchestration ----------------

def _samples_of_pair(pid):
    g, sidA, sidB = PAIRS[pid]
    return g, sidA, sidB


def kernel(**inputs):
    inputs = {k: np.asarray(v) for k, v in inputs.items()}
    x = {}        # sid-keyed raw inputs per granularity
    w1 = {}
    w2 = {}
    for gi, g in enumerate(GS):
        for n in range(5):
            x[(g, 0, n)] = inputs['sparse_' + g][n]
            x[(g, 1, n)] = inputs['dense_' + g][n]
        w1[g] = inputs['w1_' + g].astype(F32)
        w2[g] = inputs['w2_' + g].astype(F32)
    ntiles = 16                    # 32 rows per core band
    npairs = len(PAIRS)
    cores = list(range(NCORES))

    # ---- launch 1 inputs ----
    S_full = {}
    for pid, (g, sidA, sidB) in enumerate(PAIRS):
        KC = COPIES[g] * CH[g]
        SA = build_S_sample(x[(g,) + sidA])
        SB = build_S_sample(x[(g,) + sidB])
        S_full[pid] = np.concatenate([SA, SB], axis=0)  # [2KC, 258, 258]
    wblk = {pid: build_wblks(g, w1[g][:, :, :, :], w1[g])
            for pid, (g, _, _) in enumerate(PAIRS)}
    # NB: both halves use the same w1 per granularity (weights shared per g).

    in_maps1 = []
    for c in cores:
        im = {}
        r0 = c * 32
        for pid in range(npairs):
            im[f'S{pid}'] = np.ascontiguousarray(
                S_full[pid][:, r0:r0 + 34, :])
            im[f'W{pid}'] = wblk[pid]
        in_maps1.append(im)

    nc1 = build_l1(new_nc(NCORES), [{'g': g} for (g, _, _) in PAIRS], ntiles)
    r1 = bass_utils.run_bass_kernel_spmd(nc1, in_maps1, cores)
    res1 = r1.results

    # ---- host: BN1 stats ----
    P = H * W
    sum_y = {}
    sum_y2 = {}
    for pid, (g, sidA, sidB) in enumerate(PAIRS):
        acc = np.zeros((128, 32), np.float64)
        for c in cores:
            acc += res1[c]['sacc'][pid].astype(np.float64)
        sy = acc[:, 0:16].sum(1)
        sy2 = acc[:, 16:32].sum(1)
        for rows, sid in ((slice(0, 64), sidA), (slice(64, 128), sidB)):
            sum_y[(g,) + sid] = sy[rows]
            sum_y2[(g,) + sid] = sy2[rows]

    mu1 = {}
    a1 = {}
    for g in GS:
        for n in range(5):
            sid = (g, 0, n)
            m = sum_y[sid] / P
            v = sum_y2[sid] / P - m * m
            mu1[sid] = m
            a1[sid] = 1.0 / np.sqrt(v + EPS)
        sy = sum(sum_y[(g, 1, n)] for n in range(5))
        sy2 = sum(sum_y2[(g, 1, n)] for n in range(5))
        m = sy / (5 * P)
        v = sy2 / (5 * P) - m * m
        for n in range(5):
            mu1[(g, 1, n)] = m
            a1[(g, 1, n)] = 1.0 / np.sqrt(v + EPS)

    bmu1 = np.zeros((128, npairs), F32)
    for pid, (g, sidA, sidB) in enumerate(PAIRS):
        bmu1[0:64, pid] = -mu1[(g,) + sidA]
        bmu1[64:128, pid] = -mu1[(g,) + sidB]

    # ---- launch 2 ----
    in_maps2 = [{'y': res1[c]['y'], 'bmu1': bmu1} for c in cores]
    nc2 = build_l2(new_nc(NCORES), [{'g': g} for (g, _, _) in PAIRS], ntiles)
    r2 = bass_utils.run_bass_kernel_spmd(nc2, in_maps2, cores)
    res2 = r2.results

    # ---- host: BN2 stats via M ----
    Msum = {}
    m1sum = {}
    for pid, (g, sidA, sidB) in enumerate(PAIRS):
        Macc = np.zeros((128, 128), np.float64)
        m1acc = np.zeros((128,), np.float64)
        for c in cores:
            Macc += res2[c]['Mout'][pid].astype(np.float64)
            m1acc += res2[c]['m1out'][pid][:, 0].astype(np.float64)
        Msum[(g,) + sidA] = Macc[0:64, 0:64]
        Msum[(g,) + sidB] = Macc[64:128, 64:128]
        m1sum[(g,) + sidA] = m1acc[0:64]
        m1sum[(g,) + sidB] = m1acc[64:128]

    w2p = {}    # folded bf16 w2' per sample, [128,64]
    for g in GS:
        for path in (0, 1):
            for n in range(5):
                sid = (g, path, n)
                w2p[sid] = (w2[g].astype(BF16).astype(F32)
                            * a1[sid][None, :].astype(F32)).astype(BF16)

    muz = {}
    az = {}
    for g in GS:
        for n in range(5):
            sid = (g, 0, n)
            wf = w2p[sid].astype(np.float64)
            mz = wf @ (m1sum[sid] / P)
            ez2 = np.einsum('ok,kl,ol->o', wf, Msum[sid] / P, wf)
            vz = ez2 - mz * mz
            muz[sid] = mz
            az[sid] = 1.0 / np.sqrt(vz + EPS)
        Mb = sum(Msum[(g, 1, n)] for n in range(5))
        mb = sum(m1sum[(g, 1, n)] for n in range(5))
        wf = w2p[(g, 1, 0)].astype(np.float64)
        mz = wf @ (mb / (5 * P))
        ez2 = np.einsum('ok,kl,ol->o', wf, Mb / (5 * P), wf)
        vz = ez2 - mz * mz
        for n in range(5):
            muz[(g, 1, n)] = mz
            az[(g, 1, n)] = 1.0 / np.sqrt(vz + EPS)

    # ---- launch 3 inputs ----
    w2p_in = np.zeros((npairs, 128, 256), BF16)
    bmuz = np.zeros((128, 2 * npairs), F32)
    for pid, (g, sidA, sidB) in enumerate(PAIRS):
        w2p_in[pid, 0:64, 0:128] = w2p[(g,) + sidA].T       # lhsT rows=ic
        w2p_in[pid, 64:128, 128:256] = w2p[(g,) + sidB].T
        bmuz[:, 2 * pid] = -muz[(g,) + sidA]
        bmuz[:, 2 * pid + 1] = -muz[(g,) + sidB]

    in_maps3 = [{'h1': res2[c]['h1'], 'w2p': w2p_in, 'bmuz': bmuz}
                for c in cores]
    nc3 = build_l3(new_nc(NCORES), [{'g': g} for (g, _, _) in PAIRS], ntiles)
    r3 = bass_utils.run_bass_kernel_spmd(nc3, in_maps3, cores)
    res3 = r3.results

    # ---- host: embeddings + loss tail ----
    emb = {}
    for pid, (g, sidA, sidB) in enumerate(PAIRS):
        acc = np.zeros((128, 32), np.float64)
        for c in cores:
            acc += res3[c]['gacc'][pid].astype(np.float64)
        gA = acc[:, 0:16].sum(1)
        gB = acc[:, 16:32].sum(1)
        emb[(g,) + sidA] = gA / P * az[(g,) + sidA]
        emb[(g,) + sidB] = gB / P * az[(g,) + sidB]

    Qe = np.stack([[emb[(g, 0, n)] for n in range(5)] for g in GS])
    Ke = np.stack([[emb[(g, 1, n)] for n in range(5)] for g in GS])

    def norm(v):
        n = np.maximum(np.linalg.norm(v, axis=-1, keepdims=True), 1e-12)
        return v / n
    Q = norm(Qe)
    K = norm(Ke)
    S = np.einsum('gid,gjd->gij', Q, K)
    pos = np.einsum('gii->gi', S).copy()
    cav = S.shape[1]
    S_m = np.where(np.eye(cav, dtype=bool)[None], NEG, S)
    X = np.einsum('gid,hid->gih', Q, K)
    X_m = np.where(np.eye(3, dtype=bool)[:, None, :], NEG, X)
    logits = np.concatenate([pos[..., None], S_m, X_m], axis=-1) / TEMP
    mx = logits.max(-1, keepdims=True)
    lse = mx[..., 0] + np.log(np.exp(logits - mx).sum(-1))
    loss_tab = -logits[..., 0] + lse
    dm = inputs['decision_mask']
    cnt = np.stack([(dm == gg + 1).sum((1, 2)) for gg in range(3)]
                   ).astype(np.float64)
    cnt = np.pad(cnt, ((0, 0), (1, 0)))
    tot = cnt.sum()
    loss = (cnt * loss_tab).sum() / max(tot, 1.0) if tot > 0 else 0.0
    kernel.exec_ns = [rr.exec_time_ns for rr in (r1, r2, r3)]
    return np.float32(loss)


# ---------------- CoreSim self-test (small worklist, 1 core) ----------------

def _selftest():
    from concourse.bass_interp import CoreSim
    rng = np.random.RandomState(0)
    ntiles = 2
    wl = [{'g': g} for g in GS]
    xs = {g: rng.randn(2, CH[g], H, W).astype(F32) * 1.0 for g in GS}
    w1s = {g: rng.randn(64, CH[g], 3, 3).astype(F32) * 0.05 for g in GS}

    # numpy expectation of y band (rows 0..2*ntiles-1)
    def conv(xx, ww):
        C = xx.shape[0]
        xb = xx.astype(BF16).astype(F32)
        wb = ww.astype(BF16).astype(F32)
        xp = np.zeros((C, H + 2, W + 2), F32)
        xp[:, 1:-1, 1:-1] = xb
        out = np.zeros((64, H, W), F32)
        for dy in range(3):
            for dx in range(3):
                out += np.einsum('oc,chw->ohw', wb[:, :, dy, dx],
                                 xp[:, dy:dy + H, dx:dx + W])
        return out

    in_map = {}
    exp_y = []
    for pid, g in enumerate(GS):
        S = np.concatenate([build_S_sample(xs[g][0]),
                            build_S_sample(xs[g][1])], 0)
        in_map[f'S{pid}'] = np.ascontiguousarray(S[:, 0:2 * ntiles + 2, :])
        in_map[f'W{pid}'] = build_wblks(g, w1s[g], w1s[g])
        ya = conv(xs[g][0], w1s[g])[:, 0:2 * ntiles, :].reshape(64, -1)
        yb = conv(xs[g][1], w1s[g])[:, 0:2 * ntiles, :].reshape(64, -1)
        exp_y.append(np.concatenate([ya, yb], 0))
    exp_y = np.stack(exp_y)  # [3,128,ntiles*512]

    nc = build_l1(new_nc(1), wl, ntiles)
    sim = CoreSim(nc, trace=False)
    for k, v in in_map.items():
        sim.tensor(k)[:] = v
    sim.simulate(check_with_hw=False)
    got_y = np.asarray(sim.tensor('y')).astype(F32)
    err = np.abs(got_y - exp_y.astype(BF16).astype(F32)).max()
    scale = np.abs(exp_y).max()
    print(f'L1 y maxabs diff={err:.4e} (scale {scale:.2f})')
    sacc = np.asarray(sim.tensor('sacc'))
    exp_sy = exp_y.astype(BF16).astype(F32).sum(2)
    got_sy = sacc[:, :, 0:ntiles].sum(2)
    print(f'L1 sum_y rel={np.abs(got_sy-exp_sy).max()/np.abs(exp_sy).max():.3e}')
    exp_sy2 = (exp_y.astype(np.float64) ** 2).sum(2)
    got_sy2 = sacc[:, :, 16:16 + ntiles].sum(2)
    print(f'L1 sum_y2 rel={np.abs(got_sy2-exp_sy2).max()/exp_sy2.max():.3e}')

    # ---- L2 ----
    yb16 = got_y.astype(BF16)
    bmu1 = (rng.randn(128, 3) * 0.1).astype(F32)
    nc2 = build_l2(new_nc(1), wl, ntiles)
    sim2 = CoreSim(nc2, trace=False)
    sim2.tensor('y')[:] = yb16
    sim2.tensor('bmu1')[:] = bmu1
    sim2.simulate(check_with_hw=False)
    h1 = np.maximum(yb16.astype(F32) + bmu1.T[:, :, None], 0).astype(BF16).astype(F32)
    expM = np.einsum('pax,pbx->pab', h1, h1)
    gotM = np.asarray(sim2.tensor('Mout'))
    print(f'L2 M rel={np.abs(gotM-expM).max()/np.abs(expM).max():.3e}')
    expm1 = h1.sum(2)
    gotm1 = np.asarray(sim2.tensor('m1out'))[:, :, 0]
    print(f'L2 m1 rel={np.abs(gotm1-expm1).max()/np.abs(expm1).max():.3e}')

    # ---- L3 ----
    w2p_in = np.zeros((3, 128, 256), BF16)
    rng2 = np.random.RandomState(1)
    w2s = {g: rng2.randn(128, 64).astype(F32) * 0.05 for g in GS}
    for pid, g in enumerate(GS):
        w2p_in[pid, 0:64, 0:128] = w2s[g].T.astype(BF16)
        w2p_in[pid, 64:128, 128:256] = w2s[g].T.astype(BF16)
    bmuz = (rng2.randn(128, 6) * 0.05).astype(F32)
    nc3 = build_l3(new_nc(1), wl, ntiles)
    sim3 = CoreSim(nc3, trace=False)
    sim3.tensor('h1')[:] = np.asarray(sim2.tensor('h1'))
    sim3.tensor('w2p')[:] = w2p_in
    sim3.tensor('bmuz')[:] = bmuz
    sim3.simulate(check_with_hw=False)
    gotg = np.asarray(sim3.tensor('gacc'))
    for pid, g in enumerate(GS):
        w2b = w2s[g].astype(BF16).astype(F32)
        zA = w2b @ h1[pid, 0:64]
        zB = w2b @ h1[pid, 64:128]
        eA = np.maximum(zA + bmuz[:, 2 * pid:2 * pid + 1], 0).sum(1)
        eB = np.maximum(zB + bmuz[:, 2 * pid + 1:2 * pid + 2], 0).sum(1)
        gA = gotg[pid, :, 0:ntiles].sum(1)
        gB = gotg[pid, :, 16:16 + ntiles].sum(1)
        print(f'L3 p{pid} relA={np.abs(gA-eA).max()/np.abs(eA).max():.3e} '
              f'relB={np.abs(gB-eB).max()/np.abs(eB).max():.3e}')


if __name__ == '__main__':
    if '--selftest' in sys.argv:
        _selftest()
